# revision 2
# baseline (speedup 1.0000x reference)
"""Trainium2 kernel for CondensedLinearFineGrainedSparseOp:
    out[b,s,o] = sum_k x[b,s,k] * weight[o,k] + bias[o]
with x [8, 2048, 4096] f32, weight [4096, 4096] f32 (90% zeros, stored
dense), bias [4096] f32 -> out [8, 2048, 4096] f32.

Strategy: data-parallel shard over tokens (B*S = 16384 -> 2048 per core)
across 8 NeuronCores; weight/bias replicated. The unstructured 10%
sparsity is not exploitable on the 128x128 PE array (any >=8x8 block of
the mask is nonempty with overwhelming probability), so each core runs a
dense [2048 x 4096 x 4096] GEMM in bf16 with fp32 PSUM accumulation.

Per-core kernel: out[t,o] with t on PSUM partitions. Stationary operand =
x^T tile [128k, 128t]; moving operand = W^T tile [128k, 512o]. o is
processed in 4 blocks of 1024; each o-block's 32 W^T k-tiles are SBUF
resident (double-buffered across blocks, 128KB/partition) while x^T
streams per t-tile (re-read once per o-block). Bias is pre-replicated
across 128 partitions on host and added during PSUM->SBUF eviction.

HBM traffic/core ~130MB (~0.43ms) < PE dense compute ~0.87ms -> compute
bound at the bf16 PE roofline.
"""

import os

import numpy as np
import ml_dtypes

import concourse.mybir as mybir
import concourse.tile as tile
from concourse import bacc
from concourse.bass import ts
from concourse.bass_utils import run_bass_kernel_spmd

P = 128
NCORES = 8
B, S, DIN, DOUT = 8, 2048, 4096, 4096
T = B * S // NCORES          # tokens per core
KT = DIN // P                # 32 contraction tiles
NT = T // P                  # 16 token tiles per core
OBLK = 1024                  # o-block (SBUF-resident W slice)
NOB = DOUT // OBLK           # 4 o-blocks
NBANK = OBLK // 512          # 2 PSUM banks per (o-block, t-tile)

BF16 = mybir.dt.bfloat16
F32 = mybir.dt.float32

_NC = None
LAST_RESULT = None


def _build_nc():
    nc = bacc.Bacc("TRN2", target_bir_lowering=False, debug=False)
    xt = nc.dram_tensor("xt", [DIN, T], BF16, kind="ExternalInput")
    wt = nc.dram_tensor("wt", [DIN, DOUT], BF16, kind="ExternalInput")
    bias = nc.dram_tensor("bias_rep", [P, DOUT], F32, kind="ExternalInput")
    out = nc.dram_tensor("out", [T, DOUT], F32, kind="ExternalOutput")

    with tile.TileContext(nc) as tc:
        with (
            tc.tile_pool(name="wpool", bufs=2 * KT) as wpool,
            tc.tile_pool(name="xpool", bufs=3) as xpool,
            tc.tile_pool(name="bpool", bufs=1) as bpool,
            tc.tile_pool(name="opool", bufs=4) as opool,
            tc.tile_pool(name="psum", bufs=8, space="PSUM") as psum_pool,
        ):
            bias_sb = bpool.tile([P, DOUT], F32)
            nc.sync.dma_start(bias_sb[:], bias.ap())

            for ob in range(NOB):
                # W^T k-tiles for this o-block; 2*KT slots in the pool
                # double-buffer the next block's stream behind this one.
                w_tiles = []
                for k in range(KT):
                    wtile = wpool.tile([P, OBLK], BF16, tag="w")
                    nc.sync.dma_start(
                        wtile[:], wt.ap()[ts(k, P), ts(ob, OBLK)]
                    )
                    w_tiles.append(wtile)

                for t in range(NT):
                    # x^T tile [p, ks, t]: one DMA, element (ks*128+p, t0+i)
                    xtile = xpool.tile([P, KT, P], BF16, tag="x")
                    src = xt.ap()[:, ts(t, P)].rearrange(
                        "(ks p) t -> p ks t", p=P
                    )
                    nc.sync.dma_start(xtile[:], src)

                    accs = [
                        psum_pool.tile([P, 512], F32, tag="acc", name="acc")
                        for _ in range(NBANK)
                    ]
                    for k in range(KT):
                        for b in range(NBANK):
                            nc.tensor.matmul(
                                accs[b][:],
                                xtile[:, k, :],                  # stationary
                                w_tiles[k][:, ts(b, 512)],       # moving
                                start=(k == 0),
                                stop=(k == KT - 1),
                            )
                    osb = opool.tile([P, OBLK], F32, tag="o")
                    for b in range(NBANK):
                        nc.vector.tensor_add(
                            osb[:, ts(b, 512)],
                            accs[b][:],
                            bias_sb[:, ts(ob * NBANK + b, 512)],
                        )
                    nc.sync.dma_start(
                        out.ap()[ts(t, P), ts(ob, OBLK)], osb[:]
                    )

    nc.compile()
    return nc


def kernel(x, weight, bias):
    global _NC, LAST_RESULT
    if _NC is None:
        _NC = _build_nc()

    X = np.ascontiguousarray(x.reshape(B * S, DIN))
    wt = weight.T.astype(ml_dtypes.bfloat16)          # [k, o] bf16
    bias_rep = np.ascontiguousarray(
        np.broadcast_to(bias.astype(np.float32), (P, DOUT))
    )
    in_maps = []
    for c in range(NCORES):
        xt_c = X[c * T : (c + 1) * T].T.astype(ml_dtypes.bfloat16)
        in_maps.append({"xt": xt_c, "wt": wt, "bias_rep": bias_rep})

    last_err = None
    for _attempt in range(2):
        try:
            res = run_bass_kernel_spmd(_NC, in_maps, list(range(NCORES)))
            break
        except Exception as e:  # transient NRT device errors: retry once
            last_err = e
    else:
        raise last_err
    LAST_RESULT = res

    out = np.concatenate([res.results[c]["out"] for c in range(NCORES)], axis=0)
    return out.reshape(B, S, DOUT).astype(np.float32, copy=False)


# revision 4
# speedup vs baseline: 1.0059x; 1.0059x over previous
"""Trainium2 kernel for CondensedLinearFineGrainedSparseOp:
    out[b,s,o] = sum_k x[b,s,k] * weight[o,k] + bias[o]
with x [8, 2048, 4096] f32, weight [4096, 4096] f32 (90% zeros, stored
dense), bias [4096] f32 -> out [8, 2048, 4096] f32.

Strategy: data-parallel shard over tokens (B*S = 16384 -> 2048 per core)
across 8 NeuronCores; weight/bias replicated. The unstructured 10%
sparsity is not exploitable on the 128x128 PE array (any >=8x8 block of
the mask is nonempty with overwhelming probability), so each core runs a
dense [2048 x 4096 x 4096] GEMM in bf16 with fp32 PSUM accumulation.

Per-core kernel: out[t,o] with t on PSUM partitions. Stationary operand =
x^T tile [128k, 128t]; moving operand = W^T tile [128k, 512o]. o is
processed in 4 blocks of 1024; each o-block's 32 W^T k-tiles are SBUF
resident (double-buffered across blocks, 128KB/partition) while x^T
streams per t-tile (re-read once per o-block). Bias is pre-replicated
across 128 partitions on host and added during PSUM->SBUF eviction.

HBM traffic/core ~130MB (~0.43ms) < PE dense compute ~0.87ms -> compute
bound at the bf16 PE roofline.
"""

import os

import numpy as np
import ml_dtypes

import concourse.mybir as mybir
import concourse.tile as tile
from concourse import bacc
from concourse.bass import ts
from concourse.bass_utils import run_bass_kernel_spmd

P = 128
NCORES = 8
B, S, DIN, DOUT = 8, 2048, 4096, 4096
T = B * S // NCORES          # tokens per core
KT = DIN // P                # 32 contraction tiles
NT = T // P                  # 16 token tiles per core
OBLK = 1024                  # o-block (SBUF-resident W slice)
NOB = DOUT // OBLK           # 4 o-blocks
NBANK = OBLK // 512          # 2 PSUM banks per (o-block, t-tile)

BF16 = mybir.dt.bfloat16
F32 = mybir.dt.float32

_NC = None
LAST_RESULT = None


def _build_nc():
    nc = bacc.Bacc("TRN2", target_bir_lowering=False, debug=False)
    xt = nc.dram_tensor("xt", [DIN, T], BF16, kind="ExternalInput")
    wt = nc.dram_tensor("wt", [DIN, DOUT], BF16, kind="ExternalInput")
    bias = nc.dram_tensor("bias_rep", [P, DOUT], F32, kind="ExternalInput")
    out = nc.dram_tensor("out", [T, DOUT], F32, kind="ExternalOutput")

    with tile.TileContext(nc) as tc:
        with (
            tc.tile_pool(name="wpool", bufs=2 * KT) as wpool,
            tc.tile_pool(name="xpool", bufs=3) as xpool,
            tc.tile_pool(name="bpool", bufs=1) as bpool,
            tc.tile_pool(name="opool", bufs=4) as opool,
            tc.tile_pool(name="psum", bufs=8, space="PSUM") as psum_pool,
        ):
            # bias via SWDGE: keeps the sync HWDGE queue free for the
            # latency-critical first W-block stream
            bias_sb = bpool.tile([P, DOUT], F32)
            nc.gpsimd.dma_start(bias_sb[:], bias.ap())

            for ob in range(NOB):
                # W^T k-tiles for this o-block; 2*KT slots in the pool
                # double-buffer the next block's stream behind this one.
                w_tiles = []
                for k in range(KT):
                    wtile = wpool.tile([P, OBLK], BF16, tag="w")
                    nc.sync.dma_start(
                        wtile[:], wt.ap()[ts(k, P), ts(ob, OBLK)]
                    )
                    w_tiles.append(wtile)

                for t in range(NT):
                    # x^T tile [p, ks, t]: one DMA, element (ks*128+p, t0+i)
                    xtile = xpool.tile([P, KT, P], BF16, tag="x")
                    src = xt.ap()[:, ts(t, P)].rearrange(
                        "(ks p) t -> p ks t", p=P
                    )
                    # scalar (ACT) HWDGE queue: decouple x stream from the
                    # W stream on sync so neither delays the other
                    nc.scalar.dma_start(xtile[:], src)

                    accs = [
                        psum_pool.tile([P, 512], F32, tag="acc", name="acc")
                        for _ in range(NBANK)
                    ]
                    for k in range(KT):
                        for b in range(NBANK):
                            nc.tensor.matmul(
                                accs[b][:],
                                xtile[:, k, :],                  # stationary
                                w_tiles[k][:, ts(b, 512)],       # moving
                                start=(k == 0),
                                stop=(k == KT - 1),
                            )
                    osb = opool.tile([P, OBLK], F32, tag="o")
                    for b in range(NBANK):
                        nc.vector.tensor_add(
                            osb[:, ts(b, 512)],
                            accs[b][:],
                            bias_sb[:, ts(ob * NBANK + b, 512)],
                        )
                    nc.sync.dma_start(
                        out.ap()[ts(t, P), ts(ob, OBLK)], osb[:]
                    )

    nc.compile()
    return nc


def kernel(x, weight, bias):
    global _NC, LAST_RESULT
    if _NC is None:
        _NC = _build_nc()

    X = np.ascontiguousarray(x.reshape(B * S, DIN))
    wt = weight.T.astype(ml_dtypes.bfloat16)          # [k, o] bf16
    bias_rep = np.ascontiguousarray(
        np.broadcast_to(bias.astype(np.float32), (P, DOUT))
    )
    in_maps = []
    for c in range(NCORES):
        xt_c = X[c * T : (c + 1) * T].T.astype(ml_dtypes.bfloat16)
        in_maps.append({"xt": xt_c, "wt": wt, "bias_rep": bias_rep})

    last_err = None
    for _attempt in range(2):
        try:
            res = run_bass_kernel_spmd(_NC, in_maps, list(range(NCORES)))
            break
        except Exception as e:  # transient NRT device errors: retry once
            last_err = e
    else:
        raise last_err
    LAST_RESULT = res

    out = np.concatenate([res.results[c]["out"] for c in range(NCORES)], axis=0)
    return out.reshape(B, S, DOUT).astype(np.float32, copy=False)


# revision 7
# speedup vs baseline: 1.0166x; 1.0106x over previous
"""Trainium2 kernel for CondensedLinearFineGrainedSparseOp:
    out[b,s,o] = sum_k x[b,s,k] * weight[o,k] + bias[o]
with x [8, 2048, 4096] f32, weight [4096, 4096] f32 (90% zeros, stored
dense), bias [4096] f32 -> out [8, 2048, 4096] f32.

Strategy: data-parallel shard over tokens (B*S = 16384 -> 2048 per core)
across 8 NeuronCores; weight/bias replicated. The unstructured 10%
sparsity is not exploitable on the 128x128 PE array (any >=8x8 block of
the mask is nonempty with overwhelming probability), so each core runs a
dense [2048 x 4096 x 4096] GEMM in bf16 with fp32 PSUM accumulation.

Per-core kernel: out[t,o] with t on PSUM partitions. Stationary operand =
x^T tile [128k, 128t]; moving operand = W^T tile [128k, 512o]. o is
processed in 4 blocks of 1024; each o-block's 32 W^T k-tiles are SBUF
resident (double-buffered across blocks, 128KB/partition) while x^T
streams per t-tile (re-read once per o-block). Bias is pre-replicated
across 128 partitions on host and added during PSUM->SBUF eviction.

HBM traffic/core ~130MB (~0.43ms) < PE dense compute ~0.87ms -> compute
bound at the bf16 PE roofline.
"""

import os

import numpy as np
import ml_dtypes

import concourse.mybir as mybir
import concourse.tile as tile
from concourse import bacc
from concourse.bass import ts
from concourse.bass_utils import run_bass_kernel_spmd

P = 128
NCORES = 8
B, S, DIN, DOUT = 8, 2048, 4096, 4096
T = B * S // NCORES          # tokens per core
KT = DIN // P                # 32 contraction tiles
NT = T // P                  # 16 token tiles per core
OBLK = 1024                  # o-block (SBUF-resident W slice)
NOB = DOUT // OBLK           # 4 o-blocks
NBANK = OBLK // 512          # 2 PSUM banks per (o-block, t-tile)

BF16 = mybir.dt.bfloat16
F32 = mybir.dt.float32

_NC = None
LAST_RESULT = None


def _build_nc():
    nc = bacc.Bacc("TRN2", target_bir_lowering=False, debug=False)
    # x pre-tiled on host to the exact SBUF image of each t-tile:
    # xt[t, p, ks, i] = x[t*128+i, ks*128+p] -> each t-tile DMA is one
    # fully linear 1MB read (per-partition 8KB contiguous)
    xt = nc.dram_tensor("xt", [NT, P, KT, P], BF16, kind="ExternalInput")
    wt = nc.dram_tensor("wt", [DIN, DOUT], BF16, kind="ExternalInput")
    bias = nc.dram_tensor("bias_rep", [P, DOUT], F32, kind="ExternalInput")
    out = nc.dram_tensor("out", [T, DOUT], F32, kind="ExternalOutput")

    with tile.TileContext(nc) as tc:
        with (
            tc.tile_pool(name="wpool", bufs=2 * KT) as wpool,
            tc.tile_pool(name="xpool", bufs=3) as xpool,
            tc.tile_pool(name="bpool", bufs=1) as bpool,
            tc.tile_pool(name="opool", bufs=4) as opool,
            tc.tile_pool(name="psum", bufs=8, space="PSUM") as psum_pool,
        ):
            # bias via SWDGE: keeps the sync HWDGE queue free for the
            # latency-critical first W-block stream
            bias_sb = bpool.tile([P, DOUT], F32)
            nc.gpsimd.dma_start(bias_sb[:], bias.ap())

            for ob in range(NOB):
                # W^T k-tiles for this o-block; 2*KT slots in the pool
                # double-buffer the next block's stream behind this one.
                w_tiles = []
                for k in range(KT):
                    wtile = wpool.tile([P, OBLK], BF16, tag="w")
                    nc.sync.dma_start(
                        wtile[:], wt.ap()[ts(k, P), ts(ob, OBLK)]
                    )
                    w_tiles.append(wtile)

                for t in range(NT):
                    # x^T tile [p, ks, t]: one DMA, element (ks*128+p, t0+i)
                    xtile = xpool.tile([P, KT, P], BF16, tag="x")
                    # scalar (ACT) HWDGE queue: decouple x stream from the
                    # W stream on sync so neither delays the other
                    nc.scalar.dma_start(xtile[:], xt.ap()[t])

                    accs = [
                        psum_pool.tile([P, 512], F32, tag="acc", name="acc")
                        for _ in range(NBANK)
                    ]
                    for k in range(KT):
                        for b in range(NBANK):
                            nc.tensor.matmul(
                                accs[b][:],
                                xtile[:, k, :],                  # stationary
                                w_tiles[k][:, ts(b, 512)],       # moving
                                start=(k == 0),
                                stop=(k == KT - 1),
                            )
                    osb = opool.tile([P, OBLK], F32, tag="o")
                    for b in range(NBANK):
                        nc.vector.tensor_add(
                            osb[:, ts(b, 512)],
                            accs[b][:],
                            bias_sb[:, ts(ob * NBANK + b, 512)],
                        )
                    nc.sync.dma_start(
                        out.ap()[ts(t, P), ts(ob, OBLK)], osb[:]
                    )

    nc.compile()
    return nc


def kernel(x, weight, bias):
    global _NC, LAST_RESULT
    if _NC is None:
        _NC = _build_nc()

    X = np.ascontiguousarray(x.reshape(B * S, DIN))
    wt = weight.T.astype(ml_dtypes.bfloat16)          # [k, o] bf16
    bias_rep = np.ascontiguousarray(
        np.broadcast_to(bias.astype(np.float32), (P, DOUT))
    )
    in_maps = []
    for c in range(NCORES):
        xc = X[c * T : (c + 1) * T].astype(ml_dtypes.bfloat16)
        # [t-tile, p(=k%128), ks, i(=token%128)]
        xt_c = np.ascontiguousarray(
            xc.reshape(NT, P, KT, P).transpose(0, 3, 2, 1)
        )
        in_maps.append({"xt": xt_c, "wt": wt, "bias_rep": bias_rep})

    last_err = None
    for _attempt in range(2):
        try:
            res = run_bass_kernel_spmd(_NC, in_maps, list(range(NCORES)))
            break
        except Exception as e:  # transient NRT device errors: retry once
            last_err = e
    else:
        raise last_err
    LAST_RESULT = res

    out = np.concatenate([res.results[c]["out"] for c in range(NCORES)], axis=0)
    return out.reshape(B, S, DOUT).astype(np.float32, copy=False)
